# revision 9
# baseline (speedup 1.0000x reference)
"""Bass/Trainium2 kernel for nn_LocalAggregator (GNN message passing).

Math per batch b (hidden [64,128], adj [64,64] in {0..4}, a [4,128]):
    e_k[i,j] = leakyrelu_{0.2}( sum_d hidden[i,d]*hidden[j,d]*a[k,d] )
    alpha    = softmax_j( where(adj==k+1, e_k, -9e15) )
    out      = alpha @ hidden

Device strategy (8 cores, pure batch data-parallel, 64 batches/core,
processed in "quads" of 4 batches l = 2p+u, p,u in {0,1}):
  - e_k is SYMMETRIC in (i,j): the PSUM tile e4[(u,i),(p,k,j)] can be
    read as e4[(u,j),(p,k,i)], so masking with the host-TRANSPOSED
    one-hot of adj yields the transposed attention weights directly --
    no on-chip transposes.
  - w_all[d,(k,l,j)] = hidden^T * a_k: 4 contiguous tensor_scalar ops
    on DVE; the e-matmul rhs is a strided (k,j) view per batch.
  - Prelu (alpha=0.2) evacuates PSUM on ACT -> lr (fp16).
  - Selection: z = (lr + 40) * ind with a single scalar_tensor_tensor
    (ind one-hot from host); sum over k gives esel = lr_sel + 40
    (masked entries 0); Exp with bias=-40 turns masked entries into
    exp(-40) == 0 exactly in fp16. leakyrelu commutes with one-hot
    selection, so selecting after Prelu is exact.
  - Final matmul per batch (64-contract, tile_position diagonal) with a
    ones-column appended to hidden emits the softmax denominator s_i;
    the unnormalized rows + s are shipped fp16 and the HOST divides.
"""

import numpy as np

import concourse.bass as bass
import concourse.tile as tile
from concourse import bacc, mybir
from concourse._compat import with_exitstack
from concourse.bass_utils import run_bass_kernel_spmd

F16 = mybir.dt.float16
F32 = mybir.dt.float32
ALU = mybir.AluOpType
ACTF = mybir.ActivationFunctionType

B, N, D, K = 512, 64, 128, 4
NCORES = 8
BPC = B // NCORES          # 64 batches per core
QUADS = BPC // 4           # 16 quads of 4 batches per core
HHW = 132                  # hidden cols + ones col + pad
CIN = 256 + 2 * HHW + 512  # blob cols: hT(256) | hh(264) | ind(512)
MASKV = 40.0


@with_exitstack
def _kernel_body(ctx, tc, blob_d, aT_d, out_d):
    nc = tc.nc

    const_pool = ctx.enter_context(tc.tile_pool(name="const", bufs=1))
    in_pool = ctx.enter_context(tc.tile_pool(name="inp", bufs=3))
    work_pool = ctx.enter_context(tc.tile_pool(name="work", bufs=3))
    psum_pool = ctx.enter_context(tc.tile_pool(name="psum", bufs=3, space="PSUM"))
    opsum_pool = ctx.enter_context(tc.tile_pool(name="opsum", bufs=3, space="PSUM"))
    out_pool = ctx.enter_context(tc.tile_pool(name="outp", bufs=3))

    a_sb = const_pool.tile([128, 4], F32)          # a^T : [d, k]
    nc.sync.dma_start(out=a_sb[:], in_=aT_d[:, :])
    neg40 = const_pool.tile([128, 1], F32)         # Exp bias (un-does the +40)
    nc.vector.memset(neg40[:], -MASKV)

    for q in range(QUADS):
        blob = in_pool.tile([128, CIN], F16, tag="blob")
        nc.sync.dma_start(out=blob[:], in_=blob_d[q])
        hT = blob[:, 0:256]                   # [d, (l, j)]
        hh = blob[:, 256 : 256 + 2 * HHW]     # [(u,j), (p, c)]
        ind = blob[:, 256 + 2 * HHW : CIN]    # [(u,j), (p, k, i)] one-hot

        # ---- w_all[d, (k, l, j)] = hT * a_k (contiguous outputs) ----
        w_all = work_pool.tile([128, 1024], F16, tag="w_all")
        for k in range(K):
            nc.vector.tensor_scalar(
                w_all[:, k * 256 : (k + 1) * 256], hT,
                a_sb[:, k : k + 1], None, ALU.mult)

        # ---- e4[(u,i), (p,k,j)] : 4 matmuls, contract d=128 ----
        # rhs: batch l's (k, j) columns of k-major w_all (strided view)
        wv = w_all[:].rearrange("p (k l j) -> p l k j", k=4, l=4)
        e4 = psum_pool.tile([128, 512], F32, tag="e4")
        for l in range(4):
            p, u = l // 2, l % 2
            nc.tensor.matmul(
                e4[u * 64 : (u + 1) * 64, p * 256 : (p + 1) * 256],
                lhsT=hT[:, l * 64 : (l + 1) * 64],
                rhs=wv[:, l, :, :],
                start=True, stop=True,
                tile_position=(0, u * 64),
            )

        # ---- lr = Prelu(e4) evacuates PSUM; z = (lr+40)*ind ----
        lr = work_pool.tile([128, 512], F16, tag="lr")
        nc.scalar.activation(lr[:], e4[:], ACTF.Prelu, alpha=0.2)
        z = work_pool.tile([128, 512], F16, tag="z")
        nc.vector.scalar_tensor_tensor(
            z[:], lr[:], MASKV, ind, ALU.add, ALU.mult)

        # ---- esel[(u,j), (p,i)] = sum_k z ; w = exp(esel - 40) ----
        zv = z[:].rearrange("p (a k i) -> p a k i", a=2, k=4)
        t2 = work_pool.tile([128, 256], F16, tag="t2")
        t2v = t2[:].rearrange("p (a k i) -> p a k i", a=2, k=2)
        nc.vector.tensor_tensor(t2v, zv[:, :, 0:2, :], zv[:, :, 2:4, :], ALU.add)
        esel = work_pool.tile([128, 128], F16, tag="esel")
        eselv = esel[:].rearrange("p (a i) -> p a i", a=2)
        nc.vector.tensor_tensor(eselv, t2v[:, :, 0, :], t2v[:, :, 1, :], ALU.add)
        w = work_pool.tile([128, 128], F16, tag="w")
        nc.scalar.activation(w[:], esel[:], ACTF.Exp, bias=neg40[:, 0:1])

        # ---- out[(u,i), (p,c)] = sum_j w^T[j,i] hh[j,c]; col 128 = s_i ----
        osum = opsum_pool.tile([128, 2 * HHW], F32, tag="osum")
        for l in range(4):
            p, u = l // 2, l % 2
            nc.tensor.matmul(
                osum[u * 64 : (u + 1) * 64, p * HHW : (p + 1) * HHW],
                lhsT=w[u * 64 : (u + 1) * 64, p * 64 : (p + 1) * 64],
                rhs=hh[u * 64 : (u + 1) * 64, p * HHW : (p + 1) * HHW],
                start=True, stop=True,
                tile_position=(u * 64, u * 64),
            )

        # ---- evacuate (unnormalized rows + denominator) and store ----
        osb = out_pool.tile([128, 2 * HHW], F16, tag="osb")
        if q % 2 == 0:
            nc.scalar.activation(osb[:], osum[:], ACTF.Copy)
        else:
            nc.vector.tensor_scalar(osb[:], osum[:], 1.0, None, ALU.mult)
        nc.sync.dma_start(out=out_d[q], in_=osb[:])


def build_nc():
    nc = bacc.Bacc("TRN2", target_bir_lowering=False, debug=False)
    blob_d = nc.dram_tensor("blob", [QUADS, 128, CIN], F16,
                            kind="ExternalInput").ap()
    aT_d = nc.dram_tensor("at", [128, 4], F32, kind="ExternalInput").ap()
    out_d = nc.dram_tensor("out", [QUADS, 128, 2 * HHW], F16,
                           kind="ExternalOutput").ap()
    with tile.TileContext(nc) as tc:
        _kernel_body(tc, blob_d, aT_d, out_d)
    nc.compile()
    return nc


def prep_inputs(hidden, adj, a):
    """Host-side packing: fp16 casts, quad layouts, one-hot masks, shards."""
    hidden = np.asarray(hidden, dtype=np.float32)
    adj = np.asarray(adj)
    a = np.asarray(a, dtype=np.float32)

    h16 = hidden.astype(np.float16)                          # [B, 64, 128]

    # hT[q, d, l*64+j] = h[4q+l, j, d]
    hT = (h16.transpose(0, 2, 1)                             # [b, d, j]
          .reshape(B // 4, 4, D, N)                          # [q, l, d, j]
          .transpose(0, 2, 1, 3)                             # [q, d, l, j]
          .reshape(B // 4, D, 4 * N))

    # hh[q, u*64+j, p*132+c]: row (u,j) of pair p = h[4q+2p+u, j, :] + ones
    hh = np.zeros((B, N, HHW), dtype=np.float16)
    hh[:, :, 0:D] = h16
    hh[:, :, D] = np.float16(1.0)
    hhq = (hh.reshape(B // 4, 2, 2, N, HHW)                  # [q, p, u, j, c]
           .transpose(0, 2, 3, 1, 4)                         # [q, u, j, p, c]
           .reshape(B // 4, 2 * N, 2 * HHW))

    # ind[q, u*64+j, p*256+k*64+i] = (adj[4q+2p+u, i, j] == k+1)
    oh = (adj.transpose(0, 2, 1)[:, :, :, None]              # [b, j, i, 1]
          == np.arange(1, K + 1)[None, None, None, :])       # [b, j, i, k]
    indq = (oh.astype(np.float16)
            .reshape(B // 4, 2, 2, N, N, K)                  # [q, p, u, j, i, k]
            .transpose(0, 2, 3, 1, 5, 4)                     # [q, u, j, p, k, i]
            .reshape(B // 4, 2 * N, 2 * N * K))

    aT = np.ascontiguousarray(a.T).astype(np.float32)        # [128, 4]

    in_maps = []
    for c in range(NCORES):
        qsl = slice(c * QUADS, (c + 1) * QUADS)
        blob = np.empty((QUADS, 128, CIN), dtype=np.float16)
        blob[:, :, 0:256] = hT[qsl]
        blob[:, :, 256:256 + 2 * HHW] = hhq[qsl]
        blob[:, :, 256 + 2 * HHW:CIN] = indq[qsl]
        in_maps.append({"blob": np.ascontiguousarray(blob), "at": aT})
    return in_maps


_NC_CACHE = {}


def run_device(hidden, adj, a, **spmd_kwargs):
    if "nc" not in _NC_CACHE:
        _NC_CACHE["nc"] = build_nc()
    nc = _NC_CACHE["nc"]
    in_maps = prep_inputs(hidden, adj, a)
    res = run_bass_kernel_spmd(nc, in_maps, list(range(NCORES)), **spmd_kwargs)
    outs = []
    for c in range(NCORES):
        o = res.results[c]["out"].astype(np.float32)         # [QUADS, 128, 264]
        o = (o.reshape(QUADS, 2, N, 2, HHW)                  # [q, u, i, p, c]
             .transpose(0, 3, 1, 2, 4)                       # [q, p, u, i, c]
             .reshape(BPC, N, HHW))
        outs.append(o[:, :, 0:D] / o[:, :, D:D + 1])
    out = np.concatenate(outs, axis=0)
    return out.reshape(B, N, D).astype(np.float32), res


def kernel(hidden, adj, a):
    out, _ = run_device(hidden, adj, a)
    return out


# revision 10
# speedup vs baseline: 1.0624x; 1.0624x over previous
"""Bass/Trainium2 kernel for nn_LocalAggregator (GNN message passing).

Math per batch b (hidden [64,128], adj [64,64] in {0..4}, a [4,128]):
    e_k[i,j] = leakyrelu_{0.2}( sum_d hidden[i,d]*hidden[j,d]*a[k,d] )
    alpha    = softmax_j( where(adj==k+1, e_k, -9e15) )
    out      = alpha @ hidden

Device strategy (8 cores, pure batch data-parallel, 64 batches/core).
Batches are fused in PAIRS (2 batches = 128 nodes -> full-width 128x128
matmuls, half the matmul instructions; cross-batch terms are garbage
that the mask kills), and processed in OCTs (4 pairs = 8 batches) so
element-wise ops run on [128, 2048] tiles that amortize per-op
overheads.

Per oct (tiles: hT [d,(pair,i)], hh [j2b,(pair,d+ones)],
A [j2b,(pair,k,i)] additive mask from host, in {0,-40}):
  - w_all[d,(k,pair,i)] = hT * a_k          (4 contiguous tensor_scalar)
  - e2[j2b,(k,i)] = hT_pair^T @ w_all_pair  (1 matmul per pair, f32 PSUM;
    e_k symmetric -> tile read as [j,(k,i)] is e_k[i,j])
  - lr = Prelu(e2) evacuates PSUM on ACT (fp16)
  - esel = max_k (lr + A): the selected lr where adj==k+1 (exact),
    else <= -34; exp(esel) underflows to exactly 0 in fp16 for masked
    and cross-batch entries. (leakyrelu commutes with selection.)
  - out_pair[i,(d,s)] = w_pair^T @ [hh|1]: unnormalized rows + softmax
    denominator s_i, shipped fp16; the HOST divides.
"""

import numpy as np

import concourse.bass as bass
import concourse.tile as tile
from concourse import bacc, mybir
from concourse._compat import with_exitstack
from concourse.bass_utils import run_bass_kernel_spmd

F16 = mybir.dt.float16
F32 = mybir.dt.float32
ALU = mybir.AluOpType
ACTF = mybir.ActivationFunctionType

B, N, D, K = 512, 64, 128, 4
NCORES = 8
BPC = B // NCORES          # 64 batches per core
NOCT = BPC // 8            # 8 octs of 8 batches (4 pairs) per core
HHW = 132                  # hidden cols + ones col + pad
CIN = 512 + 4 * HHW + 2048  # blob cols: hT(512) | hh(528) | A(2048)
MASKV = -40.0


@with_exitstack
def _kernel_body(ctx, tc, blob_d, aT_d, out_d):
    nc = tc.nc

    const_pool = ctx.enter_context(tc.tile_pool(name="const", bufs=1))
    in_pool = ctx.enter_context(tc.tile_pool(name="inp", bufs=3))
    work_pool = ctx.enter_context(tc.tile_pool(name="work", bufs=3))
    psum_pool = ctx.enter_context(tc.tile_pool(name="psum", bufs=2, space="PSUM"))
    opsum_pool = ctx.enter_context(tc.tile_pool(name="opsum", bufs=2, space="PSUM"))
    out_pool = ctx.enter_context(tc.tile_pool(name="outp", bufs=3))

    a_sb = const_pool.tile([128, 4], F32)          # a^T : [d, k]
    nc.sync.dma_start(out=a_sb[:], in_=aT_d[:, :])

    for q in range(NOCT):
        blob = in_pool.tile([128, CIN], F16, tag="blob")
        nc.sync.dma_start(out=blob[:], in_=blob_d[q])
        hT = blob[:, 0:512]                       # [d, (pair, i)]
        hh = blob[:, 512 : 512 + 4 * HHW]         # [j2b, (pair, d+ones)]
        amask = blob[:, 512 + 4 * HHW : CIN]      # [j2b, (pair, k, i)]

        # ---- w_all[d, (k, pair, i)] = hT * a_k (contiguous outputs) ----
        w_all = work_pool.tile([128, 2048], F16, tag="w_all")
        for k in range(K):
            nc.vector.tensor_scalar(
                w_all[:, k * 512 : (k + 1) * 512], hT,
                a_sb[:, k : k + 1], None, ALU.mult)

        # ---- e2 per pair (PSUM f32), Prelu-evacuated to lr (fp16) ----
        # rhs: pair p's (k, i) columns of k-major w_all (strided view)
        wv = w_all[:].rearrange("p (k a i) -> p a k i", k=4, a=4)
        lr = work_pool.tile([128, 2048], F16, tag="lr")
        for duo in range(2):
            e2 = psum_pool.tile([128, 1024], F32, tag="e2")
            for pp in range(2):
                p = 2 * duo + pp
                nc.tensor.matmul(
                    e2[:, pp * 512 : (pp + 1) * 512],
                    lhsT=hT[:, p * 128 : (p + 1) * 128],
                    rhs=wv[:, p, :, :],
                    start=True, stop=True,
                )
            nc.scalar.activation(
                lr[:, duo * 1024 : (duo + 1) * 1024], e2[:],
                ACTF.Prelu, alpha=0.2)

        # ---- esel[j, (pair, i)] = max_k (lr + A) ----
        # lr columns are (pair, k, i); A's are (pair, k, i) as well.
        z = work_pool.tile([128, 2048], F16, tag="z")
        nc.vector.tensor_tensor(z[:], lr[:], amask, ALU.add)
        zv = z[:].rearrange("p (a k i) -> p a k i", a=4, k=4)
        t2 = work_pool.tile([128, 1024], F16, tag="t2")
        t2v = t2[:].rearrange("p (a k i) -> p a k i", a=4, k=2)
        nc.vector.tensor_tensor(t2v, zv[:, :, 0:2, :], zv[:, :, 2:4, :], ALU.max)
        esel = work_pool.tile([128, 512], F16, tag="esel")
        eselv = esel[:].rearrange("p (a i) -> p a i", a=4)
        nc.vector.tensor_tensor(eselv, t2v[:, :, 0, :], t2v[:, :, 1, :], ALU.max)

        # ---- w[j, (pair, i)] = exp(esel): masked entries -> exactly 0 ----
        w = work_pool.tile([128, 512], F16, tag="w")
        nc.scalar.activation(w[:], esel[:], ACTF.Exp)

        # ---- out_pair[i, (d,s)] = sum_j w[j,i] hh[j,c]; col 128 = s_i ----
        osum = opsum_pool.tile([128, 1024], F32, tag="osum")
        for p in range(4):
            nc.tensor.matmul(
                osum[:, p * 256 : p * 256 + HHW],
                lhsT=w[:, p * 128 : (p + 1) * 128],
                rhs=hh[:, p * HHW : (p + 1) * HHW],
                start=True, stop=True,
            )

        # ---- evacuate (unnormalized) to fp16 and store ----
        osb = out_pool.tile([128, 4 * HHW], F16, tag="osb")
        osbv = osb[:].rearrange("p (a c) -> p a c", a=4)
        osumv = osum[:].rearrange("p (a c) -> p a c", a=4)[:, :, 0:HHW]
        if q % 2 == 0:
            nc.scalar.activation(osbv, osumv, ACTF.Copy)
        else:
            nc.vector.tensor_scalar(osbv, osumv, 1.0, None, ALU.mult)
        nc.sync.dma_start(out=out_d[q], in_=osb[:])


def build_nc():
    nc = bacc.Bacc("TRN2", target_bir_lowering=False, debug=False)
    blob_d = nc.dram_tensor("blob", [NOCT, 128, CIN], F16,
                            kind="ExternalInput").ap()
    aT_d = nc.dram_tensor("at", [128, 4], F32, kind="ExternalInput").ap()
    out_d = nc.dram_tensor("out", [NOCT, 128, 4 * HHW], F16,
                           kind="ExternalOutput").ap()
    with tile.TileContext(nc) as tc:
        _kernel_body(tc, blob_d, aT_d, out_d)
    nc.compile()
    return nc


def prep_inputs(hidden, adj, a):
    """Host-side packing: fp16 casts, pair-fused block layouts, masks."""
    hidden = np.asarray(hidden, dtype=np.float32)
    adj = np.asarray(adj)
    a = np.asarray(a, dtype=np.float32)

    h16 = hidden.astype(np.float16)                          # [B, 64, 128]

    # hT[pg, d, v] with v = u*64+i, batch = 2*pg + u
    hT = (h16.transpose(0, 2, 1)                             # [b, d, i]
          .reshape(B // 2, 2, D, N)                          # [pg, u, d, i]
          .transpose(0, 2, 1, 3)                             # [pg, d, u, i]
          .reshape(B // 2, D, 2 * N))

    # hh[pg, v, c]: row v = h[2pg + v//64, v%64, :] + ones col
    hh = np.zeros((B // 2, 2 * N, HHW), dtype=np.float16)
    hh[:, :, 0:D] = h16.reshape(B // 2, 2 * N, D)
    hh[:, :, D] = np.float16(1.0)

    # A[pg, x, k*128+y] = 0 where block-diag transposed adj == k+1 else -40
    # (x = j2b, y = i2b; cross-batch blocks are all -40)
    at = adj.transpose(0, 2, 1)                              # at[b, j, i]
    A = np.full((B // 2, 2 * N, K, 2 * N), np.float16(MASKV), dtype=np.float16)
    ks = np.arange(1, K + 1)[None, None, None, :]            # [1,1,1,k]
    ohA = (at[0::2][:, :, :, None] == ks)                    # [pg, j, i, k]
    ohB = (at[1::2][:, :, :, None] == ks)
    ohAt = np.transpose(ohA, (0, 1, 3, 2))                   # [pg, j, k, i]
    ohBt = np.transpose(ohB, (0, 1, 3, 2))
    A[:, 0:N, :, 0:N][ohAt] = np.float16(0.0)
    A[:, N:2 * N, :, N:2 * N][ohBt] = np.float16(0.0)
    A = A.reshape(B // 2, 2 * N, K * 2 * N)

    aT = np.ascontiguousarray(a.T).astype(np.float32)        # [128, 4]

    # blob[oct, 128, CIN] per core: hT(4 pairs) | hh | A
    PPC = BPC // 2                                           # 32 pairs per core
    in_maps = []
    for c in range(NCORES):
        psl = slice(c * PPC, (c + 1) * PPC)
        hT_c = hT[psl].reshape(NOCT, 4, D, 2 * N)
        hh_c = hh[psl].reshape(NOCT, 4, 2 * N, HHW)
        A_c = A[psl].reshape(NOCT, 4, 2 * N, K * 2 * N)
        blob = np.empty((NOCT, 128, CIN), dtype=np.float16)
        blob[:, :, 0:512] = hT_c.transpose(0, 2, 1, 3).reshape(NOCT, 128, 512)
        blob[:, :, 512:512 + 4 * HHW] = (
            hh_c.transpose(0, 2, 1, 3).reshape(NOCT, 128, 4 * HHW))
        blob[:, :, 512 + 4 * HHW:CIN] = (
            A_c.transpose(0, 2, 1, 3).reshape(NOCT, 128, 2048))
        in_maps.append({"blob": np.ascontiguousarray(blob), "at": aT})
    return in_maps


_NC_CACHE = {}


def run_device(hidden, adj, a, **spmd_kwargs):
    if "nc" not in _NC_CACHE:
        _NC_CACHE["nc"] = build_nc()
    nc = _NC_CACHE["nc"]
    in_maps = prep_inputs(hidden, adj, a)
    res = run_bass_kernel_spmd(nc, in_maps, list(range(NCORES)), **spmd_kwargs)
    outs = []
    for c in range(NCORES):
        o = res.results[c]["out"].astype(np.float32)         # [NOCT, 128, 528]
        o = (o.reshape(NOCT, 2, N, 4, HHW)                   # [q, u, i, pair, c]
             .transpose(0, 3, 1, 2, 4)                       # [q, pair, u, i, c]
             .reshape(BPC, N, HHW))
        outs.append(o[:, :, 0:D] / o[:, :, D:D + 1])
    out = np.concatenate(outs, axis=0)
    return out.reshape(B, N, D).astype(np.float32), res


def kernel(hidden, adj, a):
    out, _ = run_device(hidden, adj, a)
    return out


# revision 12
# speedup vs baseline: 1.2456x; 1.1725x over previous
"""Bass/Trainium2 kernel for nn_LocalAggregator (GNN message passing).

Math per batch b (hidden [64,128], adj [64,64] in {0..4}, a [4,128]):
    e_k[i,j] = leakyrelu_{0.2}( sum_d hidden[i,d]*hidden[j,d]*a[k,d] )
    alpha    = softmax_j( where(adj==k+1, e_k, -9e15) )
    out      = alpha @ hidden

Device strategy (8 cores, pure batch data-parallel, 64 batches/core).
Batches are fused in PAIRS (2 batches = 128 nodes -> full-width 128x128
matmuls, half the matmul instructions; cross-batch terms are garbage
that the mask kills), and processed in OCTs (4 pairs = 8 batches) so
element-wise ops run on [128, 2048] tiles that amortize per-op
overheads.

Per oct (tiles: hT [d,(pair,i)], hh [j2b,(pair,d+ones)],
A [j2b,(pair,k,i)] additive mask from host, in {0,-40}):
  - w_all[d,(k,pair,i)] = hT * a_k          (4 contiguous tensor_scalar)
  - e2[j2b,(k,i)] = hT_pair^T @ w_all_pair  (1 matmul per pair, f32 PSUM;
    e_k symmetric -> tile read as [j,(k,i)] is e_k[i,j])
  - lr = Prelu(e2) evacuates PSUM on ACT (fp16)
  - esel = max_k (lr + A): the selected lr where adj==k+1 (exact),
    else <= -34; exp(esel) underflows to exactly 0 in fp16 for masked
    and cross-batch entries. (leakyrelu commutes with selection.)
  - out_pair[i,(d,s)] = w_pair^T @ [hh|1]: unnormalized rows + softmax
    denominator s_i, shipped fp16; the HOST divides.
"""

import numpy as np

import concourse.bass as bass
import concourse.tile as tile
from concourse import bacc, mybir
from concourse._compat import with_exitstack
from concourse.bass_utils import run_bass_kernel_spmd

F16 = mybir.dt.float16
F32 = mybir.dt.float32
ALU = mybir.AluOpType
ACTF = mybir.ActivationFunctionType

B, N, D, K = 512, 64, 128, 4
NCORES = 8
BPC = B // NCORES          # 64 batches per core
NOCT = BPC // 8            # 8 octs of 8 batches (4 pairs) per core
HHW = 132                  # hidden cols + ones col + pad
CIN = 512 + 4 * HHW + 2048  # blob cols: hT(512) | hh(528) | A(2048)
MASKV = -40.0


@with_exitstack
def _kernel_body(ctx, tc, blob_d, aT_d, out_d):
    nc = tc.nc

    const_pool = ctx.enter_context(tc.tile_pool(name="const", bufs=1))
    in_pool = ctx.enter_context(tc.tile_pool(name="inp", bufs=3))
    work_pool = ctx.enter_context(tc.tile_pool(name="work", bufs=4))
    psum_pool = ctx.enter_context(tc.tile_pool(name="psum", bufs=3, space="PSUM"))
    opsum_pool = ctx.enter_context(tc.tile_pool(name="opsum", bufs=2, space="PSUM"))
    out_pool = ctx.enter_context(tc.tile_pool(name="outp", bufs=3))

    a_sb = const_pool.tile([128, 4], F32)          # a^T : [d, k]
    nc.sync.dma_start(out=a_sb[:], in_=aT_d[:, :])

    for q in range(NOCT):
        # split load: compute-critical hT+hh first (sync queue), the big
        # mask on the scalar HWDGE queue (needed 2 pipeline stages later)
        blob = in_pool.tile([128, 512 + 4 * HHW], F16, tag="blob")
        nc.sync.dma_start(out=blob[:], in_=blob_d[q][:, 0 : 512 + 4 * HHW])
        amask_t = in_pool.tile([128, 2048], F16, tag="amask")
        nc.scalar.dma_start(out=amask_t[:], in_=blob_d[q][:, 512 + 4 * HHW : CIN])
        hT = blob[:, 0:512]                       # [d, (pair, i)]
        hh = blob[:, 512 : 512 + 4 * HHW]         # [j2b, (pair, d+ones)]
        amask = amask_t[:]                        # [j2b, (pair, k, i)]

        # ---- w_all[d, (k, pair, i)] = hT * a_k (contiguous outputs) ----
        w_all = work_pool.tile([128, 2048], F16, tag="w_all")
        for k in range(K):
            nc.vector.tensor_scalar(
                w_all[:, k * 512 : (k + 1) * 512], hT,
                a_sb[:, k : k + 1], None, ALU.mult)

        # ---- e2 per pair (PSUM f32), Prelu-evacuated to lr (fp16) ----
        # rhs: pair p's (k, i) columns of k-major w_all (strided view)
        wv = w_all[:].rearrange("p (k a i) -> p a k i", k=4, a=4)
        lr = work_pool.tile([128, 2048], F16, tag="lr")
        for duo in range(2):
            e2 = psum_pool.tile([128, 1024], F32, tag="e2")
            for pp in range(2):
                p = 2 * duo + pp
                nc.tensor.matmul(
                    e2[:, pp * 512 : (pp + 1) * 512],
                    lhsT=hT[:, p * 128 : (p + 1) * 128],
                    rhs=wv[:, p, :, :],
                    start=True, stop=True,
                )
            nc.scalar.activation(
                lr[:, duo * 1024 : (duo + 1) * 1024], e2[:],
                ACTF.Prelu, alpha=0.2)

        # ---- esel[j, (pair, i)] = max_k (lr + A) ----
        # lr columns are (pair, k, i); A's are (pair, k, i) as well.
        z = work_pool.tile([128, 2048], F16, tag="z")
        nc.vector.tensor_tensor(z[:], lr[:], amask, ALU.add)
        zv = z[:].rearrange("p (a k i) -> p a k i", a=4, k=4)
        t2 = work_pool.tile([128, 1024], F16, tag="t2")
        t2v = t2[:].rearrange("p (a k i) -> p a k i", a=4, k=2)
        nc.vector.tensor_tensor(t2v, zv[:, :, 0:2, :], zv[:, :, 2:4, :], ALU.max)
        esel = work_pool.tile([128, 512], F16, tag="esel")
        eselv = esel[:].rearrange("p (a i) -> p a i", a=4)
        nc.vector.tensor_tensor(eselv, t2v[:, :, 0, :], t2v[:, :, 1, :], ALU.max)

        # ---- w[j, (pair, i)] = exp(esel): masked entries -> exactly 0 ----
        w = work_pool.tile([128, 512], F16, tag="w")
        nc.scalar.activation(w[:], esel[:], ACTF.Exp)

        # ---- out_pair[i, (d,s)] = sum_j w[j,i] hh[j,c]; col 128 = s_i ----
        # per-duo PSUM tiles (1 bank each) + alternating evac engine
        osb = out_pool.tile([128, 4 * HHW], F16, tag="osb")
        for duo in range(2):
            osum = opsum_pool.tile([128, 272], F32, tag="osum")
            for pp in range(2):
                p = 2 * duo + pp
                nc.tensor.matmul(
                    osum[:, pp * 136 : pp * 136 + HHW],
                    lhsT=w[:, p * 128 : (p + 1) * 128],
                    rhs=hh[:, p * HHW : (p + 1) * HHW],
                    start=True, stop=True,
                )
            osbv = (osb[:, duo * 2 * HHW : (duo + 1) * 2 * HHW]
                    .rearrange("p (a c) -> p a c", a=2))
            osumv = osum[:].rearrange("p (a c) -> p a c", a=2)[:, :, 0:HHW]
            if duo == 0:
                nc.scalar.activation(osbv, osumv, ACTF.Copy)
            else:
                nc.vector.tensor_scalar(osbv, osumv, 1.0, None, ALU.mult)
        nc.sync.dma_start(out=out_d[q], in_=osb[:])


def build_nc():
    nc = bacc.Bacc("TRN2", target_bir_lowering=False, debug=False)
    blob_d = nc.dram_tensor("blob", [NOCT, 128, CIN], F16,
                            kind="ExternalInput").ap()
    aT_d = nc.dram_tensor("at", [128, 4], F32, kind="ExternalInput").ap()
    out_d = nc.dram_tensor("out", [NOCT, 128, 4 * HHW], F16,
                           kind="ExternalOutput").ap()
    with tile.TileContext(nc) as tc:
        _kernel_body(tc, blob_d, aT_d, out_d)
    nc.compile()
    return nc


def prep_inputs(hidden, adj, a):
    """Host-side packing: fp16 casts, pair-fused block layouts, masks."""
    hidden = np.asarray(hidden, dtype=np.float32)
    adj = np.asarray(adj)
    a = np.asarray(a, dtype=np.float32)

    h16 = hidden.astype(np.float16)                          # [B, 64, 128]

    # hT[pg, d, v] with v = u*64+i, batch = 2*pg + u
    hT = (h16.transpose(0, 2, 1)                             # [b, d, i]
          .reshape(B // 2, 2, D, N)                          # [pg, u, d, i]
          .transpose(0, 2, 1, 3)                             # [pg, d, u, i]
          .reshape(B // 2, D, 2 * N))

    # hh[pg, v, c]: row v = h[2pg + v//64, v%64, :] + ones col
    hh = np.zeros((B // 2, 2 * N, HHW), dtype=np.float16)
    hh[:, :, 0:D] = h16.reshape(B // 2, 2 * N, D)
    hh[:, :, D] = np.float16(1.0)

    # A[pg, x, k*128+y] = 0 where block-diag transposed adj == k+1 else -40
    # (x = j2b, y = i2b; cross-batch blocks are all -40)
    at = adj.transpose(0, 2, 1)                              # at[b, j, i]
    A = np.full((B // 2, 2 * N, K, 2 * N), np.float16(MASKV), dtype=np.float16)
    ks = np.arange(1, K + 1)[None, None, None, :]            # [1,1,1,k]
    ohA = (at[0::2][:, :, :, None] == ks)                    # [pg, j, i, k]
    ohB = (at[1::2][:, :, :, None] == ks)
    ohAt = np.transpose(ohA, (0, 1, 3, 2))                   # [pg, j, k, i]
    ohBt = np.transpose(ohB, (0, 1, 3, 2))
    A[:, 0:N, :, 0:N][ohAt] = np.float16(0.0)
    A[:, N:2 * N, :, N:2 * N][ohBt] = np.float16(0.0)
    A = A.reshape(B // 2, 2 * N, K * 2 * N)

    aT = np.ascontiguousarray(a.T).astype(np.float32)        # [128, 4]

    # blob[oct, 128, CIN] per core: hT(4 pairs) | hh | A
    PPC = BPC // 2                                           # 32 pairs per core
    in_maps = []
    for c in range(NCORES):
        psl = slice(c * PPC, (c + 1) * PPC)
        hT_c = hT[psl].reshape(NOCT, 4, D, 2 * N)
        hh_c = hh[psl].reshape(NOCT, 4, 2 * N, HHW)
        A_c = A[psl].reshape(NOCT, 4, 2 * N, K * 2 * N)
        blob = np.empty((NOCT, 128, CIN), dtype=np.float16)
        blob[:, :, 0:512] = hT_c.transpose(0, 2, 1, 3).reshape(NOCT, 128, 512)
        blob[:, :, 512:512 + 4 * HHW] = (
            hh_c.transpose(0, 2, 1, 3).reshape(NOCT, 128, 4 * HHW))
        blob[:, :, 512 + 4 * HHW:CIN] = (
            A_c.transpose(0, 2, 1, 3).reshape(NOCT, 128, 2048))
        in_maps.append({"blob": np.ascontiguousarray(blob), "at": aT})
    return in_maps


_NC_CACHE = {}


def run_device(hidden, adj, a, **spmd_kwargs):
    if "nc" not in _NC_CACHE:
        _NC_CACHE["nc"] = build_nc()
    nc = _NC_CACHE["nc"]
    in_maps = prep_inputs(hidden, adj, a)
    res = run_bass_kernel_spmd(nc, in_maps, list(range(NCORES)), **spmd_kwargs)
    outs = []
    for c in range(NCORES):
        o = res.results[c]["out"].astype(np.float32)         # [NOCT, 128, 528]
        o = (o.reshape(NOCT, 2, N, 4, HHW)                   # [q, u, i, pair, c]
             .transpose(0, 3, 1, 2, 4)                       # [q, pair, u, i, c]
             .reshape(BPC, N, HHW))
        outs.append(o[:, :, 0:D] / o[:, :, D:D + 1])
    out = np.concatenate(outs, axis=0)
    return out.reshape(B, N, D).astype(np.float32), res


def kernel(hidden, adj, a):
    out, _ = run_device(hidden, adj, a)
    return out


# revision 14
# speedup vs baseline: 1.2848x; 1.0314x over previous
"""Bass/Trainium2 kernel for nn_LocalAggregator (GNN message passing).

Math per batch b (hidden [64,128], adj [64,64] in {0..4}, a [4,128]):
    e_k[i,j] = leakyrelu_{0.2}( sum_d hidden[i,d]*hidden[j,d]*a[k,d] )
    alpha    = softmax_j( where(adj==k+1, e_k, -9e15) )
    out      = alpha @ hidden

Device strategy (8 cores, pure batch data-parallel, 64 batches/core).
Batches are fused in PAIRS (2 batches = 128 nodes -> full-width 128x128
matmuls, half the matmul instructions; cross-batch terms are garbage
that the mask kills), and processed in OCTs (4 pairs = 8 batches) so
element-wise ops run on [128, 2048] tiles that amortize per-op
overheads.

Per oct (tiles: hT [d,(pair,i)], hh [j2b,(pair,d+ones)],
A [j2b,(pair,k,i)] additive mask from host, in {0,-40}):
  - w_all[d,(k,pair,i)] = hT * a_k          (4 contiguous tensor_scalar)
  - e2[j2b,(k,i)] = hT_pair^T @ w_all_pair  (1 matmul per pair, f32 PSUM;
    e_k symmetric -> tile read as [j,(k,i)] is e_k[i,j])
  - lr = Prelu(e2) evacuates PSUM on ACT (fp16)
  - esel = max_k (lr + A): the selected lr where adj==k+1 (exact),
    else <= -34; exp(esel) underflows to exactly 0 in fp16 for masked
    and cross-batch entries. (leakyrelu commutes with selection.)
  - out_pair[i,(d,s)] = w_pair^T @ [hh|1]: unnormalized rows + softmax
    denominator s_i, shipped fp16; the HOST divides.
"""

import numpy as np

import concourse.bass as bass
import concourse.tile as tile
from concourse import bacc, mybir
from concourse._compat import with_exitstack
from concourse.bass_utils import run_bass_kernel_spmd

F16 = mybir.dt.float16
F32 = mybir.dt.float32
ALU = mybir.AluOpType
ACTF = mybir.ActivationFunctionType

B, N, D, K = 512, 64, 128, 4
NCORES = 8
BPC = B // NCORES          # 64 batches per core
NOCT = BPC // 8            # 8 octs of 8 batches (4 pairs) per core
HHW = 132                  # hidden cols + ones col + pad
CIN = 512 + 4 * HHW + 2048  # blob cols: hT(512) | hh(528) | A(2048)
MASKV = -40.0


@with_exitstack
def _kernel_body(ctx, tc, blob_d, am_d, aT_d, out_d):
    nc = tc.nc

    const_pool = ctx.enter_context(tc.tile_pool(name="const", bufs=1))
    in_pool = ctx.enter_context(tc.tile_pool(name="inp", bufs=3))
    work_pool = ctx.enter_context(tc.tile_pool(name="work", bufs=4))
    psum_pool = ctx.enter_context(tc.tile_pool(name="psum", bufs=3, space="PSUM"))
    opsum_pool = ctx.enter_context(tc.tile_pool(name="opsum", bufs=2, space="PSUM"))
    out_pool = ctx.enter_context(tc.tile_pool(name="outp", bufs=3))

    a_sb = const_pool.tile([128, 4], F32)          # a^T : [d, k]
    nc.sync.dma_start(out=a_sb[:], in_=aT_d[:, :])

    for q in range(NOCT):
        # split load: compute-critical hT+hh on the sync HWDGE queue; the
        # big mask ships int8 and is cast to fp16 by the SWDGE during DMA
        # (half the HBM bytes, and on the otherwise-idle gpsimd queue)
        blob = in_pool.tile([128, 512 + 4 * HHW], F16, tag="blob")
        nc.sync.dma_start(out=blob[:], in_=blob_d[q])
        amask_t = in_pool.tile([128, 2048], F16, tag="amask")
        nc.gpsimd.dma_start(out=amask_t[:], in_=am_d[q])
        hT = blob[:, 0:512]                       # [d, (pair, i)]
        hh = blob[:, 512 : 512 + 4 * HHW]         # [j2b, (pair, d+ones)]
        amask = amask_t[:]                        # [j2b, (pair, k, i)]

        # ---- w_all[d, (k, pair, i)] = hT * a_k (contiguous outputs) ----
        w_all = work_pool.tile([128, 2048], F16, tag="w_all")
        for k in range(K):
            nc.vector.tensor_scalar(
                w_all[:, k * 512 : (k + 1) * 512], hT,
                a_sb[:, k : k + 1], None, ALU.mult)

        # ---- e2 per pair (PSUM f32), Prelu-evacuated to lr (fp16) ----
        # rhs: pair p's (k, i) columns of k-major w_all (strided view)
        wv = w_all[:].rearrange("p (k a i) -> p a k i", k=4, a=4)
        lr = work_pool.tile([128, 2048], F16, tag="lr")
        for duo in range(2):
            e2 = psum_pool.tile([128, 1024], F32, tag="e2")
            for pp in range(2):
                p = 2 * duo + pp
                nc.tensor.matmul(
                    e2[:, pp * 512 : (pp + 1) * 512],
                    lhsT=hT[:, p * 128 : (p + 1) * 128],
                    rhs=wv[:, p, :, :],
                    start=True, stop=True,
                )
            nc.scalar.activation(
                lr[:, duo * 1024 : (duo + 1) * 1024], e2[:],
                ACTF.Prelu, alpha=0.2)

        # ---- esel[j, (pair, i)] = max_k (lr + A) ----
        # lr columns are (pair, k, i); A's are (pair, k, i) as well.
        z = work_pool.tile([128, 2048], F16, tag="z")
        nc.vector.tensor_tensor(z[:], lr[:], amask, ALU.add)
        zv = z[:].rearrange("p (a k i) -> p a k i", a=4, k=4)
        t2 = work_pool.tile([128, 1024], F16, tag="t2")
        t2v = t2[:].rearrange("p (a k i) -> p a k i", a=4, k=2)
        nc.vector.tensor_tensor(t2v, zv[:, :, 0:2, :], zv[:, :, 2:4, :], ALU.max)
        esel = work_pool.tile([128, 512], F16, tag="esel")
        eselv = esel[:].rearrange("p (a i) -> p a i", a=4)
        nc.vector.tensor_tensor(eselv, t2v[:, :, 0, :], t2v[:, :, 1, :], ALU.max)

        # ---- w[j, (pair, i)] = exp(esel): masked entries -> exactly 0 ----
        w = work_pool.tile([128, 512], F16, tag="w")
        nc.scalar.activation(w[:], esel[:], ACTF.Exp)

        # ---- out_pair[i, (d,s)] = sum_j w[j,i] hh[j,c]; col 128 = s_i ----
        # per-duo PSUM tiles (1 bank each) + alternating evac engine
        osb = out_pool.tile([128, 4 * HHW], F16, tag="osb")
        for duo in range(2):
            osum = opsum_pool.tile([128, 272], F32, tag="osum")
            for pp in range(2):
                p = 2 * duo + pp
                nc.tensor.matmul(
                    osum[:, pp * 136 : pp * 136 + HHW],
                    lhsT=w[:, p * 128 : (p + 1) * 128],
                    rhs=hh[:, p * HHW : (p + 1) * HHW],
                    start=True, stop=True,
                )
            osbv = (osb[:, duo * 2 * HHW : (duo + 1) * 2 * HHW]
                    .rearrange("p (a c) -> p a c", a=2))
            osumv = osum[:].rearrange("p (a c) -> p a c", a=2)[:, :, 0:HHW]
            if duo == 0:
                nc.scalar.activation(osbv, osumv, ACTF.Copy)
            else:
                nc.vector.tensor_scalar(osbv, osumv, 1.0, None, ALU.mult)
        nc.sync.dma_start(out=out_d[q], in_=osb[:])


def build_nc():
    nc = bacc.Bacc("TRN2", target_bir_lowering=False, debug=False)
    blob_d = nc.dram_tensor("blob", [NOCT, 128, 512 + 4 * HHW], F16,
                            kind="ExternalInput").ap()
    am_d = nc.dram_tensor("am", [NOCT, 128, 2048], mybir.dt.int8,
                          kind="ExternalInput").ap()
    aT_d = nc.dram_tensor("at", [128, 4], F32, kind="ExternalInput").ap()
    out_d = nc.dram_tensor("out", [NOCT, 128, 4 * HHW], F16,
                           kind="ExternalOutput").ap()
    with tile.TileContext(nc) as tc:
        _kernel_body(tc, blob_d, am_d, aT_d, out_d)
    nc.compile()
    return nc


def prep_inputs(hidden, adj, a):
    """Host-side packing: fp16 casts, pair-fused block layouts, masks."""
    hidden = np.asarray(hidden, dtype=np.float32)
    adj = np.asarray(adj)
    a = np.asarray(a, dtype=np.float32)

    h16 = hidden.astype(np.float16)                          # [B, 64, 128]

    # hT[pg, d, v] with v = u*64+i, batch = 2*pg + u
    hT = (h16.transpose(0, 2, 1)                             # [b, d, i]
          .reshape(B // 2, 2, D, N)                          # [pg, u, d, i]
          .transpose(0, 2, 1, 3)                             # [pg, d, u, i]
          .reshape(B // 2, D, 2 * N))

    # hh[pg, v, c]: row v = h[2pg + v//64, v%64, :] + ones col
    hh = np.zeros((B // 2, 2 * N, HHW), dtype=np.float16)
    hh[:, :, 0:D] = h16.reshape(B // 2, 2 * N, D)
    hh[:, :, D] = np.float16(1.0)

    # A[pg, x, k*128+y] = 0 where block-diag transposed adj == k+1 else -40
    # (x = j2b, y = i2b; cross-batch blocks are all -40)
    at = adj.transpose(0, 2, 1)                              # at[b, j, i]
    A = np.full((B // 2, 2 * N, K, 2 * N), MASKV, dtype=np.int8)
    ks = np.arange(1, K + 1)[None, None, None, :]            # [1,1,1,k]
    ohA = (at[0::2][:, :, :, None] == ks)                    # [pg, j, i, k]
    ohB = (at[1::2][:, :, :, None] == ks)
    ohAt = np.transpose(ohA, (0, 1, 3, 2))                   # [pg, j, k, i]
    ohBt = np.transpose(ohB, (0, 1, 3, 2))
    A[:, 0:N, :, 0:N][ohAt] = 0
    A[:, N:2 * N, :, N:2 * N][ohBt] = 0
    A = A.reshape(B // 2, 2 * N, K * 2 * N)

    aT = np.ascontiguousarray(a.T).astype(np.float32)        # [128, 4]

    # blob[oct, 128, CIN] per core: hT(4 pairs) | hh | A
    PPC = BPC // 2                                           # 32 pairs per core
    in_maps = []
    for c in range(NCORES):
        psl = slice(c * PPC, (c + 1) * PPC)
        hT_c = hT[psl].reshape(NOCT, 4, D, 2 * N)
        hh_c = hh[psl].reshape(NOCT, 4, 2 * N, HHW)
        A_c = A[psl].reshape(NOCT, 4, 2 * N, K * 2 * N)
        blob = np.empty((NOCT, 128, 512 + 4 * HHW), dtype=np.float16)
        blob[:, :, 0:512] = hT_c.transpose(0, 2, 1, 3).reshape(NOCT, 128, 512)
        blob[:, :, 512:512 + 4 * HHW] = (
            hh_c.transpose(0, 2, 1, 3).reshape(NOCT, 128, 4 * HHW))
        am = np.ascontiguousarray(
            A_c.transpose(0, 2, 1, 3).reshape(NOCT, 128, 2048))
        in_maps.append({"blob": np.ascontiguousarray(blob), "am": am,
                        "at": aT})
    return in_maps


_NC_CACHE = {}


def run_device(hidden, adj, a, **spmd_kwargs):
    if "nc" not in _NC_CACHE:
        _NC_CACHE["nc"] = build_nc()
    nc = _NC_CACHE["nc"]
    in_maps = prep_inputs(hidden, adj, a)
    res = run_bass_kernel_spmd(nc, in_maps, list(range(NCORES)), **spmd_kwargs)
    outs = []
    for c in range(NCORES):
        o = res.results[c]["out"].astype(np.float32)         # [NOCT, 128, 528]
        o = (o.reshape(NOCT, 2, N, 4, HHW)                   # [q, u, i, pair, c]
             .transpose(0, 3, 1, 2, 4)                       # [q, pair, u, i, c]
             .reshape(BPC, N, HHW))
        outs.append(o[:, :, 0:D] / o[:, :, D:D + 1])
    out = np.concatenate(outs, axis=0)
    return out.reshape(B, N, D).astype(np.float32), res


def kernel(hidden, adj, a):
    out, _ = run_device(hidden, adj, a)
    return out


# revision 17
# speedup vs baseline: 1.3108x; 1.0203x over previous
"""Bass/Trainium2 kernel for nn_LocalAggregator (GNN message passing).

Math per batch b (hidden [64,128], adj [64,64] in {0..4}, a [4,128]):
    e_k[i,j] = leakyrelu_{0.2}( sum_d hidden[i,d]*hidden[j,d]*a[k,d] )
    alpha    = softmax_j( where(adj==k+1, e_k, -9e15) )
    out      = alpha @ hidden

Device strategy (8 cores, pure batch data-parallel, 64 batches/core).
Batches are fused in PAIRS (2 batches = 128 nodes -> full-width 128x128
matmuls, half the matmul instructions; cross-batch terms are garbage
that the mask kills), and processed in OCTs (4 pairs = 8 batches) so
element-wise ops run on [128, 2048] tiles that amortize per-op
overheads.

Per oct (tiles: hT [d,(pair,i)], hh [j2b,(pair,d+ones)],
A [j2b,(pair,k,i)] additive mask from host, in {0,-40}):
  - w_all[d,(k,pair,i)] = hT * a_k          (4 contiguous tensor_scalar)
  - e2[j2b,(k,i)] = hT_pair^T @ w_all_pair  (1 matmul per pair, f32 PSUM;
    e_k symmetric -> tile read as [j,(k,i)] is e_k[i,j])
  - lr = Prelu(e2) evacuates PSUM on ACT (fp16)
  - esel = max_k (lr + A): the selected lr where adj==k+1 (exact),
    else <= -34; exp(esel) underflows to exactly 0 in fp16 for masked
    and cross-batch entries. (leakyrelu commutes with selection.)
  - out_pair[i,(d,s)] = w_pair^T @ [hh|1]: unnormalized rows + softmax
    denominator s_i, shipped fp16; the HOST divides.
"""

import numpy as np

import concourse.bass as bass
import concourse.tile as tile
from concourse import bacc, mybir
from concourse._compat import with_exitstack
from concourse.bass_utils import run_bass_kernel_spmd

F16 = mybir.dt.float16
F32 = mybir.dt.float32
ALU = mybir.AluOpType
ACTF = mybir.ActivationFunctionType

B, N, D, K = 512, 64, 128, 4
NCORES = 8
BPC = B // NCORES          # 64 batches per core
NOCT = BPC // 8            # 8 octs of 8 batches (4 pairs) per core
HHW = 132                  # hidden cols + ones col + pad
CIN = 512 + 4 * HHW + 2048  # blob cols: hT(512) | hh(528) | A(2048)
MASKV = -40.0


@with_exitstack
def _kernel_body(ctx, tc, blob_d, am_d, aT_d, out_d):
    nc = tc.nc

    const_pool = ctx.enter_context(tc.tile_pool(name="const", bufs=1))
    in_pool = ctx.enter_context(tc.tile_pool(name="inp", bufs=4))
    work_pool = ctx.enter_context(tc.tile_pool(name="work", bufs=4))
    psum_pool = ctx.enter_context(tc.tile_pool(name="psum", bufs=2, space="PSUM"))
    opsum_pool = ctx.enter_context(tc.tile_pool(name="opsum", bufs=2, space="PSUM"))
    out_pool = ctx.enter_context(tc.tile_pool(name="outp", bufs=3))

    a_sb = const_pool.tile([128, 4], F32)          # a^T : [d, k]
    nc.sync.dma_start(out=a_sb[:], in_=aT_d[:, :])

    for q in range(NOCT):
        # split load: compute-critical hT+hh on the sync HWDGE queue; the
        # big mask ships int8 and is cast to fp16 by the SWDGE during DMA
        # (half the HBM bytes, and on the otherwise-idle gpsimd queue)
        blob = in_pool.tile([128, 512 + 4 * HHW], F16, tag="blob")
        nc.sync.dma_start(out=blob[:], in_=blob_d[q])
        amask_t = in_pool.tile([128, 2048], F16, tag="amask")
        nc.gpsimd.dma_start(out=amask_t[:], in_=am_d[q])
        hT = blob[:, 0:512]                       # [d, (pair, i)]
        hh = blob[:, 512 : 512 + 4 * HHW]         # [j2b, (pair, d+ones)]
        amask = amask_t[:]                        # [j2b, (pair, k, i)]

        # ---- w_all[d, (k, pair, i)] = hT * a_k (contiguous outputs) ----
        w_all = work_pool.tile([128, 2048], F16, tag="w_all")
        for k in range(K):
            nc.vector.tensor_scalar(
                w_all[:, k * 512 : (k + 1) * 512], hT,
                a_sb[:, k : k + 1], None, ALU.mult)

        # ---- e2 per pair (PSUM f32), Prelu-evacuated to lr (fp16) ----
        # rhs: pair p's (k, i) columns of k-major w_all (strided view)
        wv = w_all[:].rearrange("p (k a i) -> p a k i", k=4, a=4)
        lr = work_pool.tile([128, 2048], F16, tag="lr")
        for duo in range(2):
            e2 = psum_pool.tile([128, 1024], F32, tag="e2")
            for pp in range(2):
                p = 2 * duo + pp
                nc.tensor.matmul(
                    e2[:, pp * 512 : (pp + 1) * 512],
                    lhsT=hT[:, p * 128 : (p + 1) * 128],
                    rhs=wv[:, p, :, :],
                    start=True, stop=True,
                )
            nc.scalar.activation(
                lr[:, duo * 1024 : (duo + 1) * 1024], e2[:],
                ACTF.Prelu, alpha=0.2)

        # ---- esel[j, (pair, i)] = max_k (lr + A) ----
        # lr columns are (pair, k, i); A's are (pair, k, i) as well.
        z = work_pool.tile([128, 2048], F16, tag="z")
        nc.vector.tensor_tensor(z[:], lr[:], amask, ALU.add)
        zv = z[:].rearrange("p (a k i) -> p a k i", a=4, k=4)
        t2 = work_pool.tile([128, 1024], F16, tag="t2")
        t2v = t2[:].rearrange("p (a k i) -> p a k i", a=4, k=2)
        nc.vector.tensor_tensor(t2v, zv[:, :, 0:2, :], zv[:, :, 2:4, :], ALU.max)
        esel = work_pool.tile([128, 512], F16, tag="esel")
        eselv = esel[:].rearrange("p (a i) -> p a i", a=4)
        nc.vector.tensor_tensor(eselv, t2v[:, :, 0, :], t2v[:, :, 1, :], ALU.max)

        # ---- w[j, (pair, i)] = exp(esel): masked entries -> exactly 0 ----
        w = work_pool.tile([128, 512], F16, tag="w")
        nc.scalar.activation(w[:], esel[:], ACTF.Exp)

        # ---- out_pair[i, (d,s)] = sum_j w[j,i] hh[j,c]; col 128 = s_i ----
        osb = out_pool.tile([128, 4 * HHW], F16, tag="osb")
        osum = opsum_pool.tile([128, 1024], F32, tag="osum")
        for p in range(4):
            nc.tensor.matmul(
                osum[:, p * 256 : p * 256 + HHW],
                lhsT=w[:, p * 128 : (p + 1) * 128],
                rhs=hh[:, p * HHW : (p + 1) * HHW],
                start=True, stop=True,
            )
        osbv = osb[:].rearrange("p (a c) -> p a c", a=4)
        osumv = osum[:].rearrange("p (a c) -> p a c", a=4)[:, :, 0:HHW]
        if q % 2 == 0:
            nc.scalar.activation(osbv, osumv, ACTF.Copy)
        else:
            nc.vector.tensor_scalar(osbv, osumv, 1.0, None, ALU.mult)
        nc.sync.dma_start(out=out_d[q], in_=osb[:])


def build_nc():
    nc = bacc.Bacc("TRN2", target_bir_lowering=False, debug=False)
    blob_d = nc.dram_tensor("blob", [NOCT, 128, 512 + 4 * HHW], F16,
                            kind="ExternalInput").ap()
    am_d = nc.dram_tensor("am", [NOCT, 128, 2048], mybir.dt.int8,
                          kind="ExternalInput").ap()
    aT_d = nc.dram_tensor("at", [128, 4], F32, kind="ExternalInput").ap()
    out_d = nc.dram_tensor("out", [NOCT, 128, 4 * HHW], F16,
                           kind="ExternalOutput").ap()
    with tile.TileContext(nc) as tc:
        _kernel_body(tc, blob_d, am_d, aT_d, out_d)
    nc.compile()
    return nc


def prep_inputs(hidden, adj, a):
    """Host-side packing: fp16 casts, pair-fused block layouts, masks."""
    hidden = np.asarray(hidden, dtype=np.float32)
    adj = np.asarray(adj)
    a = np.asarray(a, dtype=np.float32)

    h16 = hidden.astype(np.float16)                          # [B, 64, 128]

    # hT[pg, d, v] with v = u*64+i, batch = 2*pg + u
    hT = (h16.transpose(0, 2, 1)                             # [b, d, i]
          .reshape(B // 2, 2, D, N)                          # [pg, u, d, i]
          .transpose(0, 2, 1, 3)                             # [pg, d, u, i]
          .reshape(B // 2, D, 2 * N))

    # hh[pg, v, c]: row v = h[2pg + v//64, v%64, :] + ones col
    hh = np.zeros((B // 2, 2 * N, HHW), dtype=np.float16)
    hh[:, :, 0:D] = h16.reshape(B // 2, 2 * N, D)
    hh[:, :, D] = np.float16(1.0)

    # A[pg, x, k*128+y] = 0 where block-diag transposed adj == k+1 else -40
    # (x = j2b, y = i2b; cross-batch blocks are all -40)
    at = adj.transpose(0, 2, 1)                              # at[b, j, i]
    A = np.full((B // 2, 2 * N, K, 2 * N), MASKV, dtype=np.int8)
    ks = np.arange(1, K + 1)[None, None, None, :]            # [1,1,1,k]
    ohA = (at[0::2][:, :, :, None] == ks)                    # [pg, j, i, k]
    ohB = (at[1::2][:, :, :, None] == ks)
    ohAt = np.transpose(ohA, (0, 1, 3, 2))                   # [pg, j, k, i]
    ohBt = np.transpose(ohB, (0, 1, 3, 2))
    A[:, 0:N, :, 0:N][ohAt] = 0
    A[:, N:2 * N, :, N:2 * N][ohBt] = 0
    A = A.reshape(B // 2, 2 * N, K * 2 * N)

    aT = np.ascontiguousarray(a.T).astype(np.float32)        # [128, 4]

    # blob[oct, 128, CIN] per core: hT(4 pairs) | hh | A
    PPC = BPC // 2                                           # 32 pairs per core
    in_maps = []
    for c in range(NCORES):
        psl = slice(c * PPC, (c + 1) * PPC)
        hT_c = hT[psl].reshape(NOCT, 4, D, 2 * N)
        hh_c = hh[psl].reshape(NOCT, 4, 2 * N, HHW)
        A_c = A[psl].reshape(NOCT, 4, 2 * N, K * 2 * N)
        blob = np.empty((NOCT, 128, 512 + 4 * HHW), dtype=np.float16)
        blob[:, :, 0:512] = hT_c.transpose(0, 2, 1, 3).reshape(NOCT, 128, 512)
        blob[:, :, 512:512 + 4 * HHW] = (
            hh_c.transpose(0, 2, 1, 3).reshape(NOCT, 128, 4 * HHW))
        am = np.ascontiguousarray(
            A_c.transpose(0, 2, 1, 3).reshape(NOCT, 128, 2048))
        in_maps.append({"blob": np.ascontiguousarray(blob), "am": am,
                        "at": aT})
    return in_maps


_NC_CACHE = {}


def run_device(hidden, adj, a, **spmd_kwargs):
    if "nc" not in _NC_CACHE:
        _NC_CACHE["nc"] = build_nc()
    nc = _NC_CACHE["nc"]
    in_maps = prep_inputs(hidden, adj, a)
    res = run_bass_kernel_spmd(nc, in_maps, list(range(NCORES)), **spmd_kwargs)
    outs = []
    for c in range(NCORES):
        o = res.results[c]["out"].astype(np.float32)         # [NOCT, 128, 528]
        o = (o.reshape(NOCT, 2, N, 4, HHW)                   # [q, u, i, pair, c]
             .transpose(0, 3, 1, 2, 4)                       # [q, pair, u, i, c]
             .reshape(BPC, N, HHW))
        outs.append(o[:, :, 0:D] / o[:, :, D:D + 1])
    out = np.concatenate(outs, axis=0)
    return out.reshape(B, N, D).astype(np.float32), res


def kernel(hidden, adj, a):
    out, _ = run_device(hidden, adj, a)
    return out


# revision 18
# speedup vs baseline: 1.4346x; 1.0944x over previous
"""Bass/Trainium2 kernel for nn_LocalAggregator (GNN message passing).

Math per batch b (hidden [64,128], adj [64,64] in {0..4}, a [4,128]):
    e_k[i,j] = leakyrelu_{0.2}( sum_d hidden[i,d]*hidden[j,d]*a[k,d] )
    alpha    = softmax_j( where(adj==k+1, e_k, -9e15) )
    out      = alpha @ hidden

Device strategy (8 cores, pure batch data-parallel, 64 batches/core).
Batches are fused in PAIRS (2 batches = 128 nodes -> full-width 128x128
matmuls, half the matmul instructions; cross-batch terms are garbage
that the mask kills), and processed in OCTs (4 pairs = 8 batches) so
element-wise ops run on [128, 2048] tiles that amortize per-op
overheads.

Per oct (tiles: hT [d,(pair,i)], hh [j2b,(pair,d+ones)],
A [j2b,(pair,k,i)] additive mask from host, in {0,-40}):
  - w_all[d,(k,pair,i)] = hT * a_k          (4 contiguous tensor_scalar)
  - e2[j2b,(k,i)] = hT_pair^T @ w_all_pair  (1 matmul per pair, f32 PSUM;
    e_k symmetric -> tile read as [j,(k,i)] is e_k[i,j])
  - lr = Prelu(e2) evacuates PSUM on ACT (fp16)
  - esel = max_k (lr + A): the selected lr where adj==k+1 (exact),
    else <= -34; exp(esel) underflows to exactly 0 in fp16 for masked
    and cross-batch entries. (leakyrelu commutes with selection.)
  - out_pair[i,(d,s)] = w_pair^T @ [hh|1]: unnormalized rows + softmax
    denominator s_i, shipped fp16; the HOST divides.
"""

import numpy as np
import ml_dtypes

import concourse.bass as bass
import concourse.tile as tile
from concourse import bacc, mybir
from concourse._compat import with_exitstack
from concourse.bass_utils import run_bass_kernel_spmd

F16 = mybir.dt.float16
BF16 = mybir.dt.bfloat16
F32 = mybir.dt.float32
ALU = mybir.AluOpType
ACTF = mybir.ActivationFunctionType

B, N, D, K = 512, 64, 128, 4
NCORES = 8
BPC = B // NCORES          # 64 batches per core
NOCT = BPC // 8            # 8 octs of 8 batches (4 pairs) per core
HHW = 132                  # hidden cols + ones col + pad
CIN = 512 + 4 * HHW + 2048  # blob cols: hT(512) | hh(528) | A(2048)
MASKV = -40.0


@with_exitstack
def _kernel_body(ctx, tc, blob_d, hT_d, am_d, aT_d, out_d):
    nc = tc.nc

    const_pool = ctx.enter_context(tc.tile_pool(name="const", bufs=1))
    in_pool = ctx.enter_context(tc.tile_pool(name="inp", bufs=4))
    work_pool = ctx.enter_context(tc.tile_pool(name="work", bufs=4))
    psum_pool = ctx.enter_context(tc.tile_pool(name="psum", bufs=2, space="PSUM"))
    opsum_pool = ctx.enter_context(tc.tile_pool(name="opsum", bufs=2, space="PSUM"))
    out_pool = ctx.enter_context(tc.tile_pool(name="outp", bufs=3))

    a_sb = const_pool.tile([128, 4], F32)          # a^T : [d, k]
    nc.sync.dma_start(out=a_sb[:], in_=aT_d[:, :])

    for q in range(NOCT):
        # split load: compute-critical hT+hh on the sync HWDGE queue; the
        # big mask ships int8 and is cast to fp16 by the SWDGE during DMA
        # (half the HBM bytes, and on the otherwise-idle gpsimd queue)
        hT_t = in_pool.tile([128, 512], BF16, tag="hT")
        nc.sync.dma_start(out=hT_t[:], in_=hT_d[q])
        blob = in_pool.tile([128, 4 * HHW], F16, tag="blob")
        nc.sync.dma_start(out=blob[:], in_=blob_d[q])
        amask_t = in_pool.tile([128, 2048], F16, tag="amask")
        nc.gpsimd.dma_start(out=amask_t[:], in_=am_d[q])
        hT = hT_t[:]                              # [d, (pair, i)] bf16
        hh = blob[:, 0 : 4 * HHW]                 # [j2b, (pair, d+ones)]
        amask = amask_t[:]                        # [j2b, (pair, k, i)]

        # ---- w_all[d, (k, pair, i)] = hT * a_k (contiguous outputs) ----
        w_all = work_pool.tile([128, 2048], BF16, tag="w_all")
        for k in range(K):
            nc.vector.tensor_scalar(
                w_all[:, k * 512 : (k + 1) * 512], hT,
                a_sb[:, k : k + 1], None, ALU.mult)

        # ---- e2 per pair (PSUM f32), Prelu-evacuated to lr (fp16) ----
        # rhs: pair p's (k, i) columns of k-major w_all (strided view)
        wv = w_all[:].rearrange("p (k a i) -> p a k i", k=4, a=4)
        lr = work_pool.tile([128, 2048], F16, tag="lr")
        for duo in range(2):
            e2 = psum_pool.tile([128, 1024], F32, tag="e2")
            for pp in range(2):
                p = 2 * duo + pp
                nc.tensor.matmul(
                    e2[:, pp * 512 : (pp + 1) * 512],
                    lhsT=hT[:, p * 128 : (p + 1) * 128],
                    rhs=wv[:, p, :, :],
                    start=True, stop=True,
                )
            nc.scalar.activation(
                lr[:, duo * 1024 : (duo + 1) * 1024], e2[:],
                ACTF.Prelu, alpha=0.2)

        # ---- esel[j, (pair, i)] = max_k (lr + A) ----
        # lr columns are (pair, k, i); A's are (pair, k, i) as well.
        z = work_pool.tile([128, 2048], F16, tag="z")
        nc.vector.tensor_tensor(z[:], lr[:], amask, ALU.add)
        zv = z[:].rearrange("p (a k i) -> p a k i", a=4, k=4)
        t2 = work_pool.tile([128, 1024], F16, tag="t2")
        t2v = t2[:].rearrange("p (a k i) -> p a k i", a=4, k=2)
        nc.vector.tensor_tensor(t2v, zv[:, :, 0:2, :], zv[:, :, 2:4, :], ALU.max)
        esel = work_pool.tile([128, 512], F16, tag="esel")
        eselv = esel[:].rearrange("p (a i) -> p a i", a=4)
        nc.vector.tensor_tensor(eselv, t2v[:, :, 0, :], t2v[:, :, 1, :], ALU.max)

        # ---- w[j, (pair, i)] = exp(esel): masked entries -> exactly 0 ----
        w = work_pool.tile([128, 512], F16, tag="w")
        nc.scalar.activation(w[:], esel[:], ACTF.Exp)

        # ---- out_pair[i, (d,s)] = sum_j w[j,i] hh[j,c]; col 128 = s_i ----
        osb = out_pool.tile([128, 4 * HHW], F16, tag="osb")
        osum = opsum_pool.tile([128, 1024], F32, tag="osum")
        for p in range(4):
            nc.tensor.matmul(
                osum[:, p * 256 : p * 256 + HHW],
                lhsT=w[:, p * 128 : (p + 1) * 128],
                rhs=hh[:, p * HHW : (p + 1) * HHW],
                start=True, stop=True,
            )
        osbv = osb[:].rearrange("p (a c) -> p a c", a=4)
        osumv = osum[:].rearrange("p (a c) -> p a c", a=4)[:, :, 0:HHW]
        if q % 2 == 0:
            nc.scalar.activation(osbv, osumv, ACTF.Copy)
        else:
            nc.vector.tensor_scalar(osbv, osumv, 1.0, None, ALU.mult)
        nc.sync.dma_start(out=out_d[q], in_=osb[:])


def build_nc():
    nc = bacc.Bacc("TRN2", target_bir_lowering=False, debug=False)
    blob_d = nc.dram_tensor("blob", [NOCT, 128, 4 * HHW], F16,
                            kind="ExternalInput").ap()
    hT_d = nc.dram_tensor("hT", [NOCT, 128, 512], BF16,
                          kind="ExternalInput").ap()
    am_d = nc.dram_tensor("am", [NOCT, 128, 2048], mybir.dt.int8,
                          kind="ExternalInput").ap()
    aT_d = nc.dram_tensor("at", [128, 4], F32, kind="ExternalInput").ap()
    out_d = nc.dram_tensor("out", [NOCT, 128, 4 * HHW], F16,
                           kind="ExternalOutput").ap()
    with tile.TileContext(nc) as tc:
        _kernel_body(tc, blob_d, hT_d, am_d, aT_d, out_d)
    nc.compile()
    return nc


def prep_inputs(hidden, adj, a):
    """Host-side packing: fp16 casts, pair-fused block layouts, masks."""
    hidden = np.asarray(hidden, dtype=np.float32)
    adj = np.asarray(adj)
    a = np.asarray(a, dtype=np.float32)

    h16 = hidden.astype(np.float16)                          # [B, 64, 128]

    # hT[pg, d, v] with v = u*64+i, batch = 2*pg + u
    hT = (h16.transpose(0, 2, 1)                             # [b, d, i]
          .reshape(B // 2, 2, D, N)                          # [pg, u, d, i]
          .transpose(0, 2, 1, 3)                             # [pg, d, u, i]
          .reshape(B // 2, D, 2 * N))

    # hh[pg, v, c]: row v = h[2pg + v//64, v%64, :] + ones col
    hh = np.zeros((B // 2, 2 * N, HHW), dtype=np.float16)
    hh[:, :, 0:D] = h16.reshape(B // 2, 2 * N, D)
    hh[:, :, D] = np.float16(1.0)

    # A[pg, x, k*128+y] = 0 where block-diag transposed adj == k+1 else -40
    # (x = j2b, y = i2b; cross-batch blocks are all -40)
    at = adj.transpose(0, 2, 1)                              # at[b, j, i]
    A = np.full((B // 2, 2 * N, K, 2 * N), MASKV, dtype=np.int8)
    ks = np.arange(1, K + 1)[None, None, None, :]            # [1,1,1,k]
    ohA = (at[0::2][:, :, :, None] == ks)                    # [pg, j, i, k]
    ohB = (at[1::2][:, :, :, None] == ks)
    ohAt = np.transpose(ohA, (0, 1, 3, 2))                   # [pg, j, k, i]
    ohBt = np.transpose(ohB, (0, 1, 3, 2))
    A[:, 0:N, :, 0:N][ohAt] = 0
    A[:, N:2 * N, :, N:2 * N][ohBt] = 0
    A = A.reshape(B // 2, 2 * N, K * 2 * N)

    aT = np.ascontiguousarray(a.T).astype(np.float32)        # [128, 4]

    # blob[oct, 128, CIN] per core: hT(4 pairs) | hh | A
    PPC = BPC // 2                                           # 32 pairs per core
    in_maps = []
    for c in range(NCORES):
        psl = slice(c * PPC, (c + 1) * PPC)
        hT_c = hT[psl].reshape(NOCT, 4, D, 2 * N)
        hh_c = hh[psl].reshape(NOCT, 4, 2 * N, HHW)
        A_c = A[psl].reshape(NOCT, 4, 2 * N, K * 2 * N)
        blob = np.ascontiguousarray(
            hh_c.transpose(0, 2, 1, 3).reshape(NOCT, 128, 4 * HHW))
        hTb = np.ascontiguousarray(
            hT_c.transpose(0, 2, 1, 3).reshape(NOCT, 128, 512)
            .astype(np.float32).astype(ml_dtypes.bfloat16))
        am = np.ascontiguousarray(
            A_c.transpose(0, 2, 1, 3).reshape(NOCT, 128, 2048))
        in_maps.append({"blob": blob, "hT": hTb, "am": am, "at": aT})
    return in_maps


_NC_CACHE = {}


def run_device(hidden, adj, a, **spmd_kwargs):
    if "nc" not in _NC_CACHE:
        _NC_CACHE["nc"] = build_nc()
    nc = _NC_CACHE["nc"]
    in_maps = prep_inputs(hidden, adj, a)
    res = run_bass_kernel_spmd(nc, in_maps, list(range(NCORES)), **spmd_kwargs)
    outs = []
    for c in range(NCORES):
        o = res.results[c]["out"].astype(np.float32)         # [NOCT, 128, 528]
        o = (o.reshape(NOCT, 2, N, 4, HHW)                   # [q, u, i, pair, c]
             .transpose(0, 3, 1, 2, 4)                       # [q, pair, u, i, c]
             .reshape(BPC, N, HHW))
        outs.append(o[:, :, 0:D] / o[:, :, D:D + 1])
    out = np.concatenate(outs, axis=0)
    return out.reshape(B, N, D).astype(np.float32), res


def kernel(hidden, adj, a):
    out, _ = run_device(hidden, adj, a)
    return out
